# revision 1
# baseline (speedup 1.0000x reference)
"""MEX (log-sum-exp) 3x3 pooling kernel for Trainium2, 8-core SPMD.

Math: out[b,m,i,j] = log( (1/n) * sum_{c,dh,dw} exp(x[b,c,i+dh,j+dw] + off[m,c,dh,dw]) )
with n = C*3*3 = 576, eps = 1.

Identity used: the reference's per-pixel max-stabilization cancels exactly:
  out = log( sum_k exp(x_k + b_k) ) - log(n);  values are benign in fp32.

Per-core plan (core i handles batch images 2i, 2i+1):
  - E[p = img*64+c, h*128+w] = exp(x) fp16, streamed in chunks (ACT).
  - LT[p, dh*96 + dw*32 + img*16 + m] = exp(off + alpha) fp16, block-diagonal
    over img.
  - Superchunk s (2048 px = 16 output rows): 12 PSUM-accumulated matmuls
    (3 dh x 4 banks of 512) -> P[(dw,img,m), pix] per-dw-tap partial sums.
  - Evacuate: ONE DVE copy P[96,2048] -> batch buffer CB bf16 in SBUF.  Cost
    is free-dim-bound, so 96 rows cost the same as 32.
  - Fold (per batch of superchunks): 12 SBUF->SBUF DMAs remap the three
    32-row dw groups into F0/F1/F2[128 = 4x32 rows, FW] with the dw column
    shifts baked into src offsets.  DMA is the only partition-mover; each
    ring serializes DMAs at ~0.65us fixed cost, so batches are few and the
    12 DMAs are spread across the SWDGE and SP rings.
  - Combine: S = F0+F1+F2 via two DVE bf16 adds at 128 partitions (2x mode).
  - ACT Ln on [128, FW] (4x less FD than a 32-partition log); out-DMA on the
    ACT ring right after its Ln (no stall), to a permuted DRAM layout that
    the host decodes (host does reshape only).
"""

import math as _math
import os as _os

import numpy as np

EPS = 1.0
B, C, H, W = 16, 64, 128, 128
M = 16
BH = BW = 3
HO, WO = H - BH + 1, W - BW + 1  # 126, 126
N_TAPS = C * BH * BW  # 576
NCORES = 8
BPC = B // NCORES  # 2 images per core
HWP = H * W  # 16384 pixels per image plane
PAD = 768
SC = 2048  # superchunk pixels (4 psum banks of 512 fp32)
NSC = HWP // SC  # 8
# fold batches: (first superchunk, num superchunks)
# (first superchunk, num superchunks, fold partition-groups)
BATCHES = [(0, 5, 4), (5, 2, 4), (7, 1, 2)]
# output column offset per batch (fw = ns*SC//G each)
BOFF = [0, 2560, 3584]
OUTW = 4608

MM_DTYPE = _os.environ.get("MEX_MM_DTYPE", "f16")
W_ALPHA_LOG = {"f16": 10.0 * 0.6931471805599453}.get(MM_DTYPE, 0.0)

_BUILT = {}


def _build(mm_dtype: str):
    """Build (and cache) the Bass/Tile program shared by all 8 cores."""
    if mm_dtype in _BUILT:
        return _BUILT[mm_dtype]

    import concourse.bass as bass
    import concourse.bacc as bacc
    import concourse.tile as tile
    from concourse import mybir

    f32 = mybir.dt.float32
    bf16 = mybir.dt.bfloat16
    mdt = {
        "f32r": mybir.dt.float32r,
        "f32": f32,
        "bf16": mybir.dt.bfloat16,
        "f16": mybir.dt.float16,
    }[mm_dtype]
    w_alpha_log = 10.0 * _math.log(2.0) if mm_dtype == "f16" else 0.0
    AF = mybir.ActivationFunctionType

    nc = bacc.Bacc("TRN2", target_bir_lowering=False, debug=False)

    # Preload the ACT table set that contains BOTH exp and ln so the
    # interleaved exp/Ln stream never thrashes table loads (~1.3us each).
    from concourse.hw_specs import get_activation_tables

    _tabs = list(get_activation_tables(nc.m.arch).keys())
    _combined_id = _tabs.index("natural_log_exp_and_others")

    xd = nc.dram_tensor("x", [128, HWP], f32, kind="ExternalInput")
    wpd = nc.dram_tensor("wp", [128, 144], f32, kind="ExternalInput")
    outd = nc.dram_tensor("out", [128, OUTW], f32, kind="ExternalOutput")

    with tile.TileContext(nc) as tc:
        with (
            tc.tile_pool(name="singles", bufs=1) as singles,
            tc.tile_pool(name="xin", bufs=6) as xin,
            tc.tile_pool(name="psum", bufs=2, space="PSUM") as psum,
            tc.tile_pool(name="post", bufs=2) as post,
        ):
            # x DMAs immediately: one head chunk (exactly what superchunk 0's
            # matmuls need) issued as ACT's very first instruction, before the
            # ~2.7us activation-table load; big chunks stream at HBM rate on
            # the SP ring behind the two small weight DMAs.  Every chunk gets
            # its own buffer so nothing waits on slot recycling.
            xchunks = [2304] + [2048] * 6 + [1024, 768]
            xoff = 0
            xk_tiles = []
            for ci, npx in enumerate(xchunks):
                head = ci == 0
                Xk = xin.tile(
                    [128, npx], f32,
                    tag="Xh" if head else "Xb",
                    bufs=1 if head else 7,
                    name=f"Xk{ci}",
                )
                if ci == 0:
                    # preload the ACT table set with BOTH exp and ln (no
                    # thrash) as ACT's first instruction; weights then the
                    # head chunks lead the SP ring ahead of everything.
                    nc.scalar.add_instruction(
                        mybir.InstLoadActFuncSet(
                            name=nc.get_next_instruction_name(),
                            act_func_set_id=_combined_id,
                            ins=[],
                            outs=[],
                        )
                    )
                    Q = singles.tile([128, 144], f32)
                    nc.sync.dma_start(
                        out=Q[:, :], in_=bass.AP(wpd, 0, [[144, 128], [1, 144]])
                    )
                nc.sync.dma_start(
                    out=Xk[:, :], in_=bass.AP(xd, xoff, [[HWP, 128], [1, npx]])
                )
                xk_tiles.append((Xk, xoff, npx))
                xoff += npx

            # ---- E = exp(x); pad gets exp(0)=1 (emitted first so the PE
            # pre-warm and e_head are not blocked behind the weights chain) --
            E = singles.tile([128, HWP + PAD], mdt)
            Xpad = singles.tile([128, PAD], f32)
            nc.vector.memset(Xpad[:, :], 0.0)
            nc.scalar.activation(out=E[:, HWP:], in_=Xpad[:, :], func=AF.Exp, scale=EPS)

            QL = singles.tile([128, 288], f32)
            nc.vector.memset(QL[:, :], -80.0)
            QLv = QL[:, :].rearrange("p (dh dw i m) -> p dh dw i m", dh=3, dw=3, i=2)
            Qv = Q[:, :].rearrange("p (dh dw m) -> p dh dw m", dh=3, dw=3)
            nc.vector.tensor_scalar_add(
                out=QLv[0:64, :, :, 0, :], in0=Qv[0:64], scalar1=w_alpha_log
            )
            nc.vector.tensor_scalar_add(
                out=QLv[64:128, :, :, 1, :], in0=Qv[64:128], scalar1=w_alpha_log
            )
            LT = singles.tile([128, 288], mdt)
            nc.scalar.activation(out=LT[:, :], in_=QL[:, :], func=AF.Exp, scale=EPS)

            # batch buffers for the evacuated psum (bf16), padded so the fold
            # DMAs' +1/+2 column shifts stay in bounds.
            CB = []
            for bi, (s0, ns, G) in enumerate(BATCHES):
                cb = singles.tile([128, ns * SC + 8], bf16, name=f"CB{bi}")
                nc.vector.memset(cb[:, ns * SC : ns * SC + 8], 0.0)
                CB.append(cb)

            def emit_exp(k):
                Xk, xo, npx = xk_tiles[k]
                nc.scalar.activation(
                    out=E[:, xo : xo + npx], in_=Xk[:, :], func=AF.Exp, scale=EPS
                )

            xcum = []
            _c = 0
            for npx in xchunks:
                _c += npx
                xcum.append(_c)
            next_exp = 0

            LTd = LT[:, :].rearrange("p (dh c) -> p dh c", dh=3)
            ln_scale = 1.0 / (float(N_TAPS) * _math.exp(w_alpha_log))

            def emit_mains(s):
                P = psum.tile([96, SC], f32, tag="P")
                p0 = s * SC
                for dh in range(3):
                    lhsT = LTd[:, dh, :]
                    for b4 in range(SC // 512):
                        base = p0 + dh * W + b4 * 512
                        nc.tensor.matmul(
                            P[:, b4 * 512 : (b4 + 1) * 512],
                            lhsT,
                            E[:, base : base + 512],
                            start=(dh == 0),
                            stop=(dh == 2),
                        )
                return P

            fold_tiles = {}

            def emit_fold(bi, engines):
                """3*G SBUF->SBUF fold DMAs for batch bi, spread over rings."""
                s0, ns, G = BATCHES[bi]
                fw = ns * SC // G
                cb = CB[bi]
                F0 = post.tile([32 * G, fw], bf16, tag="F0", name=f"F0_{bi}")
                F1 = post.tile([32 * G, fw], bf16, tag="F1", name=f"F1_{bi}")
                F2 = post.tile([32 * G, fw], bf16, tag="F2", name=f"F2_{bi}")
                i = 0
                for q in range(G):
                    for g, Fg in enumerate((F0, F1, F2)):
                        eng = engines[i % len(engines)]
                        i += 1
                        eng(
                            out=Fg[32 * q : 32 * q + 32, :],
                            in_=cb[32 * g : 32 * g + 32, fw * q + g : fw * q + g + fw],
                        )
                fold_tiles[bi] = (F0, F1, F2)

            def emit_adds(bi):
                s0, ns, G = BATCHES[bi]
                fw = ns * SC // G
                F0, F1, F2 = fold_tiles.pop(bi)
                A = post.tile([32 * G, fw], bf16, tag="A", name=f"A_{bi}", bufs=1)
                nc.vector.tensor_add(out=A[:, :], in0=F0[:, :], in1=F1[:, :])
                S = post.tile([32 * G, fw], bf16, tag="S", name=f"S_{bi}", bufs=1)
                nc.vector.tensor_add(out=S[:, :], in0=A[:, :], in1=F2[:, :])
                return S

            def emit_ln_out(bi, S):
                """Ln on ACT then the out DMA."""
                s0, ns, G = BATCHES[bi]
                fw = ns * SC // G
                LG = post.tile([32 * G, fw], f32, tag="LG", name=f"LG_{bi}", bufs=3)
                nc.scalar.activation(out=LG[:, :], in_=S[:, :], func=AF.Ln, scale=ln_scale)
                out_dma = [nc.gpsimd.dma_start, nc.gpsimd.dma_start, nc.sync.dma_start][bi]
                out_dma(
                    out=bass.AP(outd, BOFF[bi], [[OUTW, 32 * G], [1, fw]]),
                    in_=LG[:, :],
                )

            # batch id for each superchunk + offset within the batch
            sc2batch = {}
            for bi, (s0, ns, G) in enumerate(BATCHES):
                for k in range(ns):
                    sc2batch[s0 + k] = (bi, k)

            gdma = nc.gpsimd.dma_start
            sdma = nc.sync.dma_start
            # PE pre-warm: dummy matmuls on the (already-exp'd) pad columns
            # into the first psum buffer; mains(0) starts with start=True so
            # the garbage is cleared.  Keeps HAM at K=8/8 from the start.
            Pw = psum.tile([96, SC], f32, tag="P", name="Pwarm")
            for _i in range(8):
                nc.tensor.matmul(
                    Pw[:, 0:512],
                    LTd[:, 0, :],
                    E[:, HWP : HWP + 512],
                    start=True,
                    stop=True,
                )
            for s in range(NSC):
                # ensure E coverage for this superchunk's matmuls first
                needed = HWP if s >= 5 else SC * (s + 1) + 256
                while next_exp < len(xchunks) and (
                    next_exp == 0 or xcum[next_exp - 1] < needed
                ):
                    emit_exp(next_exp)
                    next_exp += 1
                P = emit_mains(s)
                bi, k = sc2batch[s]
                cb = CB[bi]
                dst = cb[0:96, k * SC : (k + 1) * SC]
                if s == 5:
                    nc.scalar.copy(out=dst, in_=P[:, :])
                elif s == 7:
                    # last copy: halves on DVE || ACT; split at 1028 so the
                    # fold DMAs for quarters 0-1 (reading cols <= 1025)
                    # depend only on the DVE half and start earlier
                    h = 1028
                    nc.vector.tensor_copy(
                        out=cb[0:96, k * SC : k * SC + h], in_=P[:, 0:h]
                    )
                    nc.scalar.copy(
                        out=cb[0:96, k * SC + h : (k + 1) * SC], in_=P[:, h:SC]
                    )
                else:
                    nc.vector.tensor_copy(out=dst, in_=P[:, :])
                if s == 4:
                    # SP only: FIFO places these after the x stream, so the
                    # fold transfers can never steal engine time from it.
                    emit_fold(0, [sdma])
                if s == 6:
                    emit_fold(1, [sdma, gdma])
                if s == 7:
                    emit_fold(2, [sdma, gdma])
            S_list = [emit_adds(bi) for bi in range(len(BATCHES))]
            for bi, Sb in enumerate(S_list):
                emit_ln_out(bi, Sb)

    nc.compile()
    _BUILT[mm_dtype] = nc
    return nc


def _prep_inputs(x, offsets):
    x = np.ascontiguousarray(np.asarray(x), dtype=np.float32)
    off = np.asarray(offsets, dtype=np.float32).reshape(M, C, BH, BW)
    # wp[c, dh*48 + dw*16 + m] = off[m, c, dh, dw]
    wp = np.ascontiguousarray(np.transpose(off, (1, 2, 3, 0)).reshape(64, 144))
    wp = np.ascontiguousarray(np.concatenate([wp, wp], axis=0))  # [128, 144]
    in_maps = [
        {"x": np.ascontiguousarray(x[BPC * i : BPC * (i + 1)]).reshape(128, HWP), "wp": wp}
        for i in range(NCORES)
    ]
    return in_maps


def _decode(raw):
    """raw [128, OUTW] per core -> [BPC, M, HO, WO].

    Batch bi at superchunk s0, G fold groups, fw = ns*SC//G:
    raw[32q+p, BOFF[bi] + j] = out[p, row 16*s0 + (fw//128)*q + j//128,
    col j%128] for q < G.
    """
    a = np.empty((32, 128, 128), dtype=raw.dtype)
    for bi, (s0, ns, G) in enumerate(BATCHES):
        fw = ns * SC // G
        rows_per_q = fw // 128  # rows per partition group
        blk = raw[0 : 32 * G, BOFF[bi] : BOFF[bi] + fw].reshape(
            G, 32, rows_per_q, 128
        )
        r0 = 16 * s0
        a[:, r0 : r0 + G * rows_per_q, :] = blk.transpose(1, 0, 2, 3).reshape(
            32, G * rows_per_q, 128
        )
    return a[:, :HO, :WO].reshape(BPC, M, HO, WO)


def kernel(x, offsets):
    from concourse.bass_utils import run_bass_kernel_spmd

    nc = _build(MM_DTYPE)
    in_maps = _prep_inputs(x, offsets)
    res = run_bass_kernel_spmd(nc, in_maps, core_ids=list(range(NCORES)))
    out = np.empty((B, M, HO, WO), dtype=np.float32)
    for i in range(NCORES):
        out[BPC * i : BPC * (i + 1)] = _decode(res.results[i]["out"])
    return out



# revision 2
# speedup vs baseline: 1.2104x; 1.2104x over previous
"""MEX (log-sum-exp) 3x3 pooling kernel for Trainium2, 8-core SPMD. v2.

Math: out[b,m,i,j] = log( (1/n) * sum_{c,dh,dw} exp(x[b,c,i+dh,j+dw] + off[m,c,dh,dw]) )
with n = C*3*3 = 576, eps = 1.  Max-stabilization cancels exactly; a fixed
alpha shift keeps fp8/f16 intermediates in range.

Per-core plan (core i handles batch images 2i, 2i+1):
  - x uploaded as f16 (halves the HBM stream vs fp32 baseline).
  - E[p, px] = exp(x) in fp8e4m3 (ACT), LT = exp(off + alpha) fp8,
    block-diagonal over the two images; 96 columns = 3 dw x (2 img x 16 m).
  - Superchunk s (2048 px): per 512-px psum bank, ONE fp8 DoubleRow matmul
    covers taps (dh=0, dh=1) via k-tile pairs at rhs stride W=128, plus ONE
    regular fp8 matmul for dh=2: 8 matmuls/superchunk vs 12 for the f16
    3-tap scheme (DR measured at the same 216 ns per 512-col matmul).
  - Evac: DVE copy PSUM -> CB f16 (partial sums fit f16 at alpha=4).
  - Fold: ONE SBUF->SBUF DMA per batch (4D access pattern does the
    (dw-shift + 4-quarter partition pack) in a single instruction) into
    F[128, 3*fw]; two adds (F0+F1 on GpSimd, +F2 on DVE) -> S[128, fw].
  - Ln on ACT reads S, scale folds exp(-alpha)/n; f16 out, host upcasts.
"""

import math as _math
import os as _os

import numpy as np

EPS = 1.0
B, C, H, W = 16, 64, 128, 128
M = 16
BH = BW = 3
HO, WO = H - BH + 1, W - BW + 1  # 126, 126
N_TAPS = C * BH * BW  # 576
NCORES = 8
BPC = B // NCORES  # 2 images per core
HWP = H * W  # 16384 pixels per image plane
PAD = 768
SC = 2048  # superchunk pixels (4 psum banks of 512 fp32)
NSC = HWP // SC  # 8
ALPHA = 4.0

# batches: (first superchunk, num superchunks); G=4 fold groups each
BATCHES = [(0, 2), (2, 2), (4, 2), (6, 1)]  # sc7 -> direct-Ln path
FWS = [ns * SC // 4 for _, ns in BATCHES]  # 1024 each
BOFF = [0]
for _fw in FWS[:-1]:
    BOFF.append(BOFF[-1] + _fw)
DIRECT_OFF = sum(FWS)  # 3584
OUTW = DIRECT_OFF + 512  # 4096

_BUILT = {}


def _build():
    """Build (and cache) the Bass/Tile program shared by all 8 cores."""
    if "nc" in _BUILT:
        return _BUILT["nc"]

    import concourse.bass as bass
    import concourse.bacc as bacc
    import concourse.tile as tile
    from concourse import mybir

    f32 = mybir.dt.float32
    f16 = mybir.dt.float16
    f8 = mybir.dt.float8e4
    AF = mybir.ActivationFunctionType
    DRM = mybir.MatmulPerfMode.DoubleRow

    nc = bacc.Bacc("TRN2", target_bir_lowering=False, debug=False)

    from concourse.hw_specs import get_activation_tables

    _tabs = list(get_activation_tables(nc.m.arch).keys())
    _combined_id = _tabs.index("natural_log_exp_and_others")

    xd = nc.dram_tensor("x", [128, HWP], f16, kind="ExternalInput")
    wpd = nc.dram_tensor("wp", [128, 144], f32, kind="ExternalInput")
    outd = nc.dram_tensor("out", [128, OUTW], f16, kind="ExternalOutput")

    ln_scale = _math.exp(-ALPHA) / float(N_TAPS)

    with tile.TileContext(nc) as tc:
        with (
            tc.tile_pool(name="singles", bufs=1) as singles,
            tc.tile_pool(name="xin", bufs=6) as xin,
            tc.tile_pool(name="psum", bufs=2, space="PSUM") as psum,
            tc.tile_pool(name="post", bufs=2) as post,
        ):
            # x chunks: head small (pipeline start), later big (8KB DMA lines)
            xchunks = [1280, 1024, 1024, 1024, 2048, 2048, 2048, 2048, 3840]
            assert sum(xchunks) == HWP
            xoff = 0
            xk_tiles = []
            for ci, npx in enumerate(xchunks):
                head = ci == 0
                Xk = xin.tile(
                    [128, npx], f16,
                    tag="Xh" if head else "Xb",
                    bufs=1 if head else 7,
                    name=f"Xk{ci}",
                )
                if ci == 0:
                    nc.scalar.add_instruction(
                        mybir.InstLoadActFuncSet(
                            name=nc.get_next_instruction_name(),
                            act_func_set_id=_combined_id,
                            ins=[],
                            outs=[],
                        )
                    )
                nc.sync.dma_start(
                    out=Xk[:, :], in_=bass.AP(xd, xoff, [[HWP, 128], [1, npx]])
                )
                if ci == 0:
                    Q = singles.tile([128, 144], f32)
                    nc.sync.dma_start(
                        out=Q[:, :], in_=bass.AP(wpd, 0, [[144, 128], [1, 144]])
                    )
                xk_tiles.append((Xk, xoff, npx))
                xoff += npx

            # ---- E = exp(x) in fp8; pad is ZERO (only affects cropped cols)
            E = singles.tile([128, HWP + PAD], f8)
            nc.vector.memset(E[:, HWP:], 0.0)
            Wdummy = singles.tile([128, 96], f8)
            nc.vector.memset(Wdummy[:, :], 0.0)

            # ---- LT = exp(off + alpha) fp8, block-diagonal over img halves
            QL = singles.tile([128, 288], f32)
            nc.vector.memset(QL[:, :], -40.0)
            QLv = QL[:, :].rearrange("p (dh dw i m) -> p dh dw i m", dh=3, dw=3, i=2)
            Qv = Q[:, :].rearrange("p (dh dw m) -> p dh dw m", dh=3, dw=3)
            nc.vector.tensor_scalar_add(
                out=QLv[0:64, :, :, 0, :], in0=Qv[0:64], scalar1=ALPHA
            )
            nc.vector.tensor_scalar_add(
                out=QLv[64:128, :, :, 1, :], in0=Qv[64:128], scalar1=ALPHA
            )
            LT = singles.tile([128, 288], f8)
            nc.scalar.activation(out=LT[:, :], in_=QL[:, :], func=AF.Exp, scale=EPS)
            # lhsT for the DR pair (dh0, dh1): [128, t=2, 96]
            LT01 = LT[:, 0:192].rearrange("p (t c) -> p t c", t=2)
            LT2 = LT[:, 192:288]
            E_ap = E[:, :]
            EFREE = HWP + PAD

            # batch buffers (f16), padded for the +1/+2 fold column shifts
            CB = []
            for bi, (s0, ns) in enumerate(BATCHES):
                cb = singles.tile([96, ns * SC + 8], f16, name=f"CB{bi}")
                nc.vector.memset(cb[:, ns * SC : ns * SC + 8], 0.0)
                CB.append(cb)

            def emit_exp(k):
                Xk, xo, npx = xk_tiles[k]
                nc.scalar.activation(
                    out=E[:, xo : xo + npx], in_=Xk[:, :], func=AF.Exp, scale=EPS
                )

            xcum = []
            _c = 0
            for npx in xchunks:
                _c += npx
                xcum.append(_c)
            next_exp = 0

            def emit_mains(s):
                P = psum.tile([96, SC], f32, tag="P")
                p0 = s * SC
                # 4 DoubleRow matmuls (dh0+dh1), then 4 regular (dh2)
                for b4 in range(4):
                    base = p0 + b4 * 512
                    rhs = bass.AP(
                        E_ap.tensor, E_ap.offset + base,
                        [[EFREE, 128], [W, 2], [1, 512]],
                    )
                    nc.tensor.matmul(
                        P[:, b4 * 512 : (b4 + 1) * 512],
                        LT01, rhs, start=True, stop=False, perf_mode=DRM,
                    )
                for b4 in range(4):
                    base = p0 + b4 * 512 + 2 * W
                    nc.tensor.matmul(
                        P[:, b4 * 512 : (b4 + 1) * 512],
                        LT2, E[:, base : base + 512], start=False, stop=True,
                    )
                return P

            def emit_fold(bi, eng):
                """ONE SBUF->SBUF DMA: dw-shift + 4-quarter partition pack."""
                s0, ns = BATCHES[bi]
                fw = FWS[bi]
                cb = CB[bi]
                Ff = post.tile([128, 3 * fw], f16, tag="F", name=f"F_{bi}")
                src_ap = cb[:, :]
                # per dw-group g, ONE DMA: dst[P=4p+q, g*fw + j] <-
                # cb[32g+p, fw*q+g+j]; src partition dim stays outermost.
                pitch = src_ap.ap[0][0]
                for g in range(3):
                    src = bass.AP(
                        src_ap.tensor,
                        src_ap.offset + 32 * g * pitch + g,
                        [[pitch, 32], [fw, 4], [1, fw]],
                    )
                    eng(out=Ff[:, g * fw : (g + 1) * fw], in_=src)
                return Ff

            def emit_adds(bi, Ff, eng1, eng2):
                fw = FWS[bi]
                A = post.tile([128, fw], f16, tag="A", name=f"A_{bi}")
                eng1(out=A[:, :], in0=Ff[:, 0:fw], in1=Ff[:, fw : 2 * fw])
                S = post.tile([128, fw], f16, tag="S", name=f"S_{bi}")
                eng2(out=S[:, :], in0=A[:, :], in1=Ff[:, 2 * fw : 3 * fw])
                return S

            def emit_ln_out(bi, S, out_dma):
                fw = FWS[bi]
                LG = post.tile([128, fw], f16, tag="LG", name=f"LG_{bi}")
                nc.scalar.activation(
                    out=LG[:, :], in_=S[:, :], func=AF.Ln, scale=ln_scale
                )
                out_dma(
                    out=bass.AP(outd, BOFF[bi], [[OUTW, 128], [1, fw]]),
                    in_=LG[:, :],
                )

            sc2batch = {}
            for bi, (s0, ns) in enumerate(BATCHES):
                for k in range(ns):
                    sc2batch[s0 + k] = (bi, k)

            gdma = nc.gpsimd.dma_start
            sdma = nc.sync.dma_start

            # PE pre-warm on the (zero) pad columns
            Pw = psum.tile([96, SC], f32, tag="P", name="Pwarm")
            for _i in range(12):
                nc.tensor.matmul(
                    Pw[:, 0:512], Wdummy[:, :], E[:, HWP : HWP + 512],
                    start=True, stop=True,
                )

            ga = nc.gpsimd.tensor_add
            va = nc.vector.tensor_add
            out_engs = [gdma, gdma, sdma, sdma]

            def emit_batch_post(bi):
                S = emit_adds(bi, folds[bi], va, va)
                emit_ln_out(bi, S, out_engs[bi])

            folds = {}
            batch_last = {s0 + ns - 1: bi for bi, (s0, ns) in enumerate(BATCHES)}
            # adds for batch bi enter the DVE FIFO after evac(key)
            post_at = {3: 0, 5: 1, 6: 2}
            for s in range(NSC - 1):
                needed = HWP if s >= NSC - 2 else SC * (s + 1) + 256
                while next_exp < len(xchunks) and (
                    next_exp == 0 or xcum[next_exp - 1] < needed
                ):
                    emit_exp(next_exp)
                    next_exp += 1
                P = emit_mains(s)
                bi, k = sc2batch[s]
                cb = CB[bi]
                nc.vector.tensor_copy(
                    out=cb[0:96, k * SC : (k + 1) * SC], in_=P[:, :]
                )
                if s in batch_last:
                    fbi = batch_last[s]
                    folds[fbi] = emit_fold(fbi, gdma if fbi < 2 else sdma)
                if s in post_at and post_at[s] is not None:
                    emit_batch_post(post_at[s])
            # ---- sc7: all 9 taps accumulate in PSUM (4 DR pairs + 1 reg
            # per bank); Ln reads PSUM directly; no evac/fold/adds.
            while next_exp < len(xchunks):
                emit_exp(next_exp)
                next_exp += 1
            ltp = LT[:, :].ap[0][0]
            lt_ap = LT[:, :]
            PW7 = psum.tile([32, SC], f32, tag="P", name="PW7")
            p7 = (NSC - 1) * SC
            # (rhs t0 offset, rhs k-stride, lhsT col offset, lhsT col stride)
            # taps: (0,1)->blocks 0,1; (128,129)->blocks 3,4;
            # (256,257)->blocks 6,7; (2,130)->blocks 2,5; reg 258->block 8
            pair_cfg = [
                (0, 1, 0, 32),
                (128, 1, 96, 32),
                (256, 1, 192, 32),
                (2, 128, 64, 96),
            ]
            for b4 in range(4):
                base = p7 + b4 * 512
                for k, (o0, dstride, wo, ws) in enumerate(pair_cfg):
                    lhsTk = bass.AP(
                        lt_ap.tensor, lt_ap.offset + wo,
                        [[ltp, 128], [ws, 2], [1, 32]],
                    )
                    rhsk = bass.AP(
                        E_ap.tensor, E_ap.offset + base + o0,
                        [[EFREE, 128], [dstride, 2], [1, 512]],
                    )
                    nc.tensor.matmul(
                        PW7[:, b4 * 512 : (b4 + 1) * 512],
                        lhsTk, rhsk, start=(k == 0), stop=False, perf_mode=DRM,
                    )
                nc.tensor.matmul(
                    PW7[:, b4 * 512 : (b4 + 1) * 512],
                    LT[:, 256:288], E[:, base + 258 : base + 258 + 512],
                    start=False, stop=True,
                )
            emit_batch_post(3)
            LG7 = post.tile([32, SC], f16, tag="LG7", bufs=1)
            nc.scalar.activation(
                out=LG7[:, :], in_=PW7[:, :], func=AF.Ln, scale=ln_scale
            )
            sdma(
                out=bass.AP(
                    outd, DIRECT_OFF, [[OUTW, 32], [32 * OUTW, 4], [1, 512]]
                ),
                in_=LG7[:, :],
            )

    nc.compile()
    _BUILT["nc"] = nc
    return nc


def _prep_inputs(x, offsets):
    x = np.asarray(x)
    off = np.asarray(offsets, dtype=np.float32).reshape(M, C, BH, BW)
    # wp[c, dh*48 + dw*16 + m] = off[m, c, dh, dw]
    wp = np.ascontiguousarray(np.transpose(off, (1, 2, 3, 0)).reshape(64, 144))
    wp = np.ascontiguousarray(np.concatenate([wp, wp], axis=0))  # [128, 144]
    in_maps = []
    for i in range(NCORES):
        xi = np.ascontiguousarray(
            x[BPC * i : BPC * (i + 1)], dtype=np.float16
        ).reshape(128, HWP)
        in_maps.append({"x": xi, "wp": wp})
    return in_maps


def _decode(raw):
    """raw [128, OUTW] f16 per core -> [BPC, M, HO, WO] f32.

    Batch bi covering superchunks [s0, s0+ns): raw[4r+q, BOFF[bi]+j] =
    out value for plane r (= img*16+m), pixel 2048*s0 + fw*q + j.
    """
    a = np.empty((32, HWP), dtype=np.float32)
    for bi, (s0, ns) in enumerate(BATCHES):
        fw = FWS[bi]
        blk = raw[:, BOFF[bi] : BOFF[bi] + fw].astype(np.float32).reshape(32, 4, fw)
        base = SC * s0
        for q in range(4):
            a[:, base + fw * q : base + fw * (q + 1)] = blk[:, q]
    # direct sc7 region: raw[32q+p, DIRECT_OFF+jj] = a[p, 7*2048 + 512q + jj]
    blk = raw[:, DIRECT_OFF : DIRECT_OFF + 512].astype(np.float32).reshape(4, 32, 512)
    for q in range(4):
        a[:, 7 * SC + 512 * q : 7 * SC + 512 * (q + 1)] = blk[q]
    a = a.reshape(32, H, W)[:, :HO, :WO]
    return a.reshape(BPC, M, HO, WO)


def kernel(x, offsets):
    from concourse.bass_utils import run_bass_kernel_spmd

    nc = _build()
    in_maps = _prep_inputs(x, offsets)
    res = run_bass_kernel_spmd(nc, in_maps, core_ids=list(range(NCORES)))
    out = np.empty((B, M, HO, WO), dtype=np.float32)
    for i in range(NCORES):
        out[BPC * i : BPC * (i + 1)] = _decode(res.results[i]["out"])
    return out


# revision 3
# speedup vs baseline: 1.2362x; 1.0213x over previous
"""MEX (log-sum-exp) 3x3 pooling kernel for Trainium2, 8-core SPMD. v2.

Math: out[b,m,i,j] = log( (1/n) * sum_{c,dh,dw} exp(x[b,c,i+dh,j+dw] + off[m,c,dh,dw]) )
with n = C*3*3 = 576, eps = 1.  Max-stabilization cancels exactly; a fixed
alpha shift keeps fp8/f16 intermediates in range.

Per-core plan (core i handles batch images 2i, 2i+1):
  - x uploaded as f16 (halves the HBM stream vs fp32 baseline).
  - E[p, px] = exp(x) in fp8e4m3 (ACT), LT = exp(off + alpha) fp8,
    block-diagonal over the two images; 96 columns = 3 dw x (2 img x 16 m).
  - Superchunk s (2048 px): per 512-px psum bank, ONE fp8 DoubleRow matmul
    covers taps (dh=0, dh=1) via k-tile pairs at rhs stride W=128, plus ONE
    regular fp8 matmul for dh=2: 8 matmuls/superchunk vs 12 for the f16
    3-tap scheme (DR measured at the same 216 ns per 512-col matmul).
  - Evac: DVE copy PSUM -> CB f16 (partial sums fit f16 at alpha=4).
  - Fold: ONE SBUF->SBUF DMA per batch (4D access pattern does the
    (dw-shift + 4-quarter partition pack) in a single instruction) into
    F[128, 3*fw]; two adds (F0+F1 on GpSimd, +F2 on DVE) -> S[128, fw].
  - Ln on ACT reads S, scale folds exp(-alpha)/n; f16 out, host upcasts.
"""

import math as _math
import os as _os

import numpy as np

EPS = 1.0
B, C, H, W = 16, 64, 128, 128
M = 16
BH = BW = 3
HO, WO = H - BH + 1, W - BW + 1  # 126, 126
N_TAPS = C * BH * BW  # 576
NCORES = 8
BPC = B // NCORES  # 2 images per core
HWP = H * W  # 16384 pixels per image plane
PAD = 768
SC = 2048  # superchunk pixels (4 psum banks of 512 fp32)
NSC = HWP // SC  # 8
ALPHA = 4.0

# batches: (first superchunk, num superchunks); G=4 fold groups each
BATCHES = [(0, 2), (2, 2), (4, 2)]  # sc6, sc7 -> direct-Ln path
FWS = [ns * SC // 4 for _, ns in BATCHES]  # 1024 each
BOFF = [0]
for _fw in FWS[:-1]:
    BOFF.append(BOFF[-1] + _fw)
DIRECT_OFF = sum(FWS)  # 3072
OUTW = DIRECT_OFF + 1024  # 4096

_BUILT = {}


def _build():
    """Build (and cache) the Bass/Tile program shared by all 8 cores."""
    if "nc" in _BUILT:
        return _BUILT["nc"]

    import concourse.bass as bass
    import concourse.bacc as bacc
    import concourse.tile as tile
    from concourse import mybir

    f32 = mybir.dt.float32
    f16 = mybir.dt.float16
    f8 = mybir.dt.float8e4
    AF = mybir.ActivationFunctionType
    DRM = mybir.MatmulPerfMode.DoubleRow

    nc = bacc.Bacc("TRN2", target_bir_lowering=False, debug=False)

    from concourse.hw_specs import get_activation_tables

    _tabs = list(get_activation_tables(nc.m.arch).keys())
    _combined_id = _tabs.index("natural_log_exp_and_others")

    xd = nc.dram_tensor("x", [128, HWP], f16, kind="ExternalInput")
    wpd = nc.dram_tensor("wp", [128, 144], f32, kind="ExternalInput")
    outd = nc.dram_tensor("out", [128, OUTW], f16, kind="ExternalOutput")

    ln_scale = _math.exp(-ALPHA) / float(N_TAPS)

    with tile.TileContext(nc) as tc:
        with (
            tc.tile_pool(name="singles", bufs=1) as singles,
            tc.tile_pool(name="xin", bufs=6) as xin,
            tc.tile_pool(name="psum", bufs=2, space="PSUM") as psum,
            tc.tile_pool(name="post", bufs=2) as post,
        ):
            # x chunks: head small (pipeline start), later big (8KB DMA lines)
            xchunks = [1280, 1024, 1024, 1024, 2048, 2048, 2048, 2048, 3840]
            assert sum(xchunks) == HWP
            xoff = 0
            xk_tiles = []
            for ci, npx in enumerate(xchunks):
                head = ci == 0
                Xk = xin.tile(
                    [128, npx], f16,
                    tag="Xh" if head else "Xb",
                    bufs=1 if head else 7,
                    name=f"Xk{ci}",
                )
                if ci == 0:
                    nc.scalar.add_instruction(
                        mybir.InstLoadActFuncSet(
                            name=nc.get_next_instruction_name(),
                            act_func_set_id=_combined_id,
                            ins=[],
                            outs=[],
                        )
                    )
                nc.sync.dma_start(
                    out=Xk[:, :], in_=bass.AP(xd, xoff, [[HWP, 128], [1, npx]])
                )
                if ci == 0:
                    Q = singles.tile([128, 144], f32)
                    nc.sync.dma_start(
                        out=Q[:, :], in_=bass.AP(wpd, 0, [[144, 128], [1, 144]])
                    )
                xk_tiles.append((Xk, xoff, npx))
                xoff += npx

            # ---- E = exp(x) in fp8; pad is ZERO (only affects cropped cols)
            E = singles.tile([128, HWP + PAD], f8)
            nc.vector.memset(E[:, HWP:], 0.0)
            Wdummy = singles.tile([128, 96], f8)
            nc.vector.memset(Wdummy[:, :], 0.0)

            # ---- LT = exp(off + alpha) fp8, block-diagonal over img halves
            QL = singles.tile([128, 288], f32)
            nc.vector.memset(QL[:, :], -40.0)
            QLv = QL[:, :].rearrange("p (dh dw i m) -> p dh dw i m", dh=3, dw=3, i=2)
            Qv = Q[:, :].rearrange("p (dh dw m) -> p dh dw m", dh=3, dw=3)
            nc.vector.tensor_scalar_add(
                out=QLv[0:64, :, :, 0, :], in0=Qv[0:64], scalar1=ALPHA
            )
            nc.vector.tensor_scalar_add(
                out=QLv[64:128, :, :, 1, :], in0=Qv[64:128], scalar1=ALPHA
            )
            LT = singles.tile([128, 288], f8)
            nc.scalar.activation(out=LT[:, :], in_=QL[:, :], func=AF.Exp, scale=EPS)
            # lhsT for the DR pair (dh0, dh1): [128, t=2, 96]
            LT01 = LT[:, 0:192].rearrange("p (t c) -> p t c", t=2)
            LT2 = LT[:, 192:288]
            E_ap = E[:, :]
            EFREE = HWP + PAD

            # batch buffers (f16), padded for the +1/+2 fold column shifts
            CB = []
            for bi, (s0, ns) in enumerate(BATCHES):
                cb = singles.tile([96, ns * SC + 8], f16, name=f"CB{bi}")
                nc.vector.memset(cb[:, ns * SC : ns * SC + 8], 0.0)
                CB.append(cb)

            def emit_exp(k):
                Xk, xo, npx = xk_tiles[k]
                nc.scalar.activation(
                    out=E[:, xo : xo + npx], in_=Xk[:, :], func=AF.Exp, scale=EPS
                )

            xcum = []
            _c = 0
            for npx in xchunks:
                _c += npx
                xcum.append(_c)
            next_exp = 0

            def emit_mains(s):
                P = psum.tile([96, SC], f32, tag="P")
                p0 = s * SC
                # 4 DoubleRow matmuls (dh0+dh1), then 4 regular (dh2)
                for b4 in range(4):
                    base = p0 + b4 * 512
                    rhs = bass.AP(
                        E_ap.tensor, E_ap.offset + base,
                        [[EFREE, 128], [W, 2], [1, 512]],
                    )
                    nc.tensor.matmul(
                        P[:, b4 * 512 : (b4 + 1) * 512],
                        LT01, rhs, start=True, stop=False, perf_mode=DRM,
                    )
                for b4 in range(4):
                    base = p0 + b4 * 512 + 2 * W
                    nc.tensor.matmul(
                        P[:, b4 * 512 : (b4 + 1) * 512],
                        LT2, E[:, base : base + 512], start=False, stop=True,
                    )
                return P

            def emit_fold(bi, eng):
                """ONE SBUF->SBUF DMA: dw-shift + 4-quarter partition pack."""
                s0, ns = BATCHES[bi]
                fw = FWS[bi]
                cb = CB[bi]
                Ff = post.tile([128, 3 * fw], f16, tag="F", name=f"F_{bi}")
                src_ap = cb[:, :]
                # per dw-group g, ONE DMA: dst[P=4p+q, g*fw + j] <-
                # cb[32g+p, fw*q+g+j]; src partition dim stays outermost.
                pitch = src_ap.ap[0][0]
                for g in range(3):
                    src = bass.AP(
                        src_ap.tensor,
                        src_ap.offset + 32 * g * pitch + g,
                        [[pitch, 32], [fw, 4], [1, fw]],
                    )
                    eng(out=Ff[:, g * fw : (g + 1) * fw], in_=src)
                return Ff

            def emit_adds(bi, Ff, eng1, eng2):
                fw = FWS[bi]
                A = post.tile([128, fw], f16, tag="A", name=f"A_{bi}")
                eng1(out=A[:, :], in0=Ff[:, 0:fw], in1=Ff[:, fw : 2 * fw])
                S = post.tile([128, fw], f16, tag="S", name=f"S_{bi}")
                eng2(out=S[:, :], in0=A[:, :], in1=Ff[:, 2 * fw : 3 * fw])
                return S

            def emit_ln_out(bi, S, out_dma):
                fw = FWS[bi]
                LG = post.tile([128, fw], f16, tag="LG", name=f"LG_{bi}")
                nc.scalar.activation(
                    out=LG[:, :], in_=S[:, :], func=AF.Ln, scale=ln_scale
                )
                out_dma(
                    out=bass.AP(outd, BOFF[bi], [[OUTW, 128], [1, fw]]),
                    in_=LG[:, :],
                )

            sc2batch = {}
            for bi, (s0, ns) in enumerate(BATCHES):
                for k in range(ns):
                    sc2batch[s0 + k] = (bi, k)

            gdma = nc.gpsimd.dma_start
            sdma = nc.sync.dma_start

            # PE pre-warm on the (zero) pad columns
            Pw = psum.tile([96, SC], f32, tag="P", name="Pwarm")
            for _i in range(12):
                nc.tensor.matmul(
                    Pw[:, 0:512], Wdummy[:, :], E[:, HWP : HWP + 512],
                    start=True, stop=True,
                )

            ga = nc.gpsimd.tensor_add
            va = nc.vector.tensor_add
            out_engs = [gdma, gdma, sdma]

            def emit_batch_post(bi):
                S = emit_adds(bi, folds[bi], va, va)
                emit_ln_out(bi, S, out_engs[bi])

            folds = {}
            batch_last = {s0 + ns - 1: bi for bi, (s0, ns) in enumerate(BATCHES)}
            # adds for batch bi enter the DVE FIFO after evac(key)
            post_at = {3: 0, 5: 1}
            for s in range(NSC - 2):
                needed = HWP if s >= NSC - 2 else SC * (s + 1) + 256
                while next_exp < len(xchunks) and (
                    next_exp == 0 or xcum[next_exp - 1] < needed
                ):
                    emit_exp(next_exp)
                    next_exp += 1
                P = emit_mains(s)
                bi, k = sc2batch[s]
                cb = CB[bi]
                nc.vector.tensor_copy(
                    out=cb[0:96, k * SC : (k + 1) * SC], in_=P[:, :]
                )
                if s in batch_last:
                    fbi = batch_last[s]
                    folds[fbi] = emit_fold(fbi, gdma if fbi < 2 else sdma)
                if s in post_at and post_at[s] is not None:
                    emit_batch_post(post_at[s])
            # ---- sc6, sc7: all 9 taps accumulate in PSUM (4 DR pairs +
            # 1 reg per bank); Ln reads PSUM directly; no evac/fold/adds.
            while next_exp < len(xchunks):
                emit_exp(next_exp)
                next_exp += 1
            emit_batch_post(2)
            ltp = LT[:, :].ap[0][0]
            lt_ap = LT[:, :]
            # (rhs t0 offset, rhs k-stride, lhsT col offset, lhsT col stride)
            # taps: (0,1)->blocks 0,1; (128,129)->blocks 3,4;
            # (256,257)->blocks 6,7; (2,130)->blocks 2,5; reg 258->block 8
            pair_cfg = [
                (0, 1, 0, 32),
                (128, 1, 96, 32),
                (256, 1, 192, 32),
                (2, 128, 64, 96),
            ]
            for di, sd in enumerate((NSC - 2, NSC - 1)):
                PW = psum.tile([32, SC], f32, tag="P", name=f"PW{sd}")
                pD = sd * SC
                for b4 in range(4):
                    base = pD + b4 * 512
                    for k, (o0, dstride, wo, ws) in enumerate(pair_cfg):
                        lhsTk = bass.AP(
                            lt_ap.tensor, lt_ap.offset + wo,
                            [[ltp, 128], [ws, 2], [1, 32]],
                        )
                        rhsk = bass.AP(
                            E_ap.tensor, E_ap.offset + base + o0,
                            [[EFREE, 128], [dstride, 2], [1, 512]],
                        )
                        nc.tensor.matmul(
                            PW[:, b4 * 512 : (b4 + 1) * 512],
                            lhsTk, rhsk, start=(k == 0), stop=False,
                            perf_mode=DRM,
                        )
                    nc.tensor.matmul(
                        PW[:, b4 * 512 : (b4 + 1) * 512],
                        LT[:, 256:288], E[:, base + 258 : base + 258 + 512],
                        start=False, stop=True,
                    )
                LGd = post.tile([32, SC], f16, tag=f"LG{sd}", bufs=1)
                nc.scalar.activation(
                    out=LGd[:, :], in_=PW[:, :], func=AF.Ln, scale=ln_scale
                )
                sdma(
                    out=bass.AP(
                        outd, DIRECT_OFF + 512 * di,
                        [[OUTW, 32], [32 * OUTW, 4], [1, 512]],
                    ),
                    in_=LGd[:, :],
                )

    nc.compile()
    _BUILT["nc"] = nc
    return nc


def _prep_inputs(x, offsets):
    x = np.asarray(x)
    off = np.asarray(offsets, dtype=np.float32).reshape(M, C, BH, BW)
    # wp[c, dh*48 + dw*16 + m] = off[m, c, dh, dw]
    wp = np.ascontiguousarray(np.transpose(off, (1, 2, 3, 0)).reshape(64, 144))
    wp = np.ascontiguousarray(np.concatenate([wp, wp], axis=0))  # [128, 144]
    in_maps = []
    for i in range(NCORES):
        xi = np.ascontiguousarray(
            x[BPC * i : BPC * (i + 1)], dtype=np.float16
        ).reshape(128, HWP)
        in_maps.append({"x": xi, "wp": wp})
    return in_maps


def _decode(raw):
    """raw [128, OUTW] f16 per core -> [BPC, M, HO, WO] f32.

    Batch bi covering superchunks [s0, s0+ns): raw[4r+q, BOFF[bi]+j] =
    out value for plane r (= img*16+m), pixel 2048*s0 + fw*q + j.
    """
    a = np.empty((32, HWP), dtype=np.float32)
    for bi, (s0, ns) in enumerate(BATCHES):
        fw = FWS[bi]
        blk = raw[:, BOFF[bi] : BOFF[bi] + fw].astype(np.float32).reshape(32, 4, fw)
        base = SC * s0
        for q in range(4):
            a[:, base + fw * q : base + fw * (q + 1)] = blk[:, q]
    # direct regions: raw[32q+p, DIRECT_OFF+512*di+jj] =
    # a[p, (6+di)*2048 + 512q + jj]
    for di, sd in enumerate((6, 7)):
        blk = raw[:, DIRECT_OFF + 512 * di : DIRECT_OFF + 512 * (di + 1)]
        blk = blk.astype(np.float32).reshape(4, 32, 512)
        for q in range(4):
            a[:, sd * SC + 512 * q : sd * SC + 512 * (q + 1)] = blk[q]
    a = a.reshape(32, H, W)[:, :HO, :WO]
    return a.reshape(BPC, M, HO, WO)


def kernel(x, offsets):
    from concourse.bass_utils import run_bass_kernel_spmd

    nc = _build()
    in_maps = _prep_inputs(x, offsets)
    res = run_bass_kernel_spmd(nc, in_maps, core_ids=list(range(NCORES)))
    out = np.empty((B, M, HO, WO), dtype=np.float32)
    for i in range(NCORES):
        out[BPC * i : BPC * (i + 1)] = _decode(res.results[i]["out"])
    return out
